# revision 29
# baseline (speedup 1.0000x reference)
"""Causal self-attention (B=4, T=2048, C=1024, H=16) on 8 TRN2 NeuronCores.

Sharding: core = 2*b + hg  (b = batch 0..3, hg = head-group 0..1, 8 heads each).
All matmul inputs are bf16 (PSUM accumulation stays fp32; ~5e-3 rel err):
  prologue: k^T and v (with an interleaved ones-column per head for softmax
            denominators) for all 16 key chunks
  main loop over query chunks n: scores^T = k^T.T @ q^T per head pair
            (row-groups 0/64 pack the two heads), exp on ACT straight from
            PSUM (no max subtraction - scores are O(1)), causal triangle via
            gpsimd affine_select, AV matmuls run two pairs behind their exp
            so the tensor queue never waits on the activation, deferred
            per-query normalization; the q^T projection for chunk n+1 and the
            output projection for chunk n-1 are interleaved as fillers.
  output:   column-split tensor parallel out-proj: the head-pair cores
            AllGather each y^T chunk in bf16 (two half-chunk AGs per chunk;
            the final chunk uses four per-head-pair AGs plus four parallel
            out-proj accumulators in AG-arrival order to minimize the tail),
            then each core computes its own 512 output channels over the full
            1024 y rows and writes out^T directly.
Host side transposes/casts x per batch on the way in and reassembles the
output on the way out.
"""
import numpy as np
from contextlib import ExitStack

import ml_dtypes

import concourse.bass as bass
from concourse import bacc, mybir
from concourse.tile import TileContext
from concourse.bass_utils import run_bass_kernel_spmd

dt = mybir.dt
AF = mybir.ActivationFunctionType
BF16 = ml_dtypes.bfloat16

B, T, C, H = 4, 2048, 1024, 16
D = 64              # head dim
HL = 8              # heads per core
CL = HL * D         # 512 local channels
NQ = T // 512       # 4 query chunks of 512
NT = T // 128       # 16 key/time chunks of 128
SCALE = 1.0 / np.sqrt(D)

_CACHE = {}


def _build_nc():
    nc = bacc.Bacc("TRN2", target_bir_lowering=False, debug=False)

    xT_e = nc.declare_dram_parameter("xT", [C, T], dt.bfloat16, isOutput=False)
    wqk_e = nc.declare_dram_parameter("wqk", [C, 2 * CL], dt.bfloat16, isOutput=False)
    wv_e = nc.declare_dram_parameter("wv", [C, CL], dt.bfloat16, isOutput=False)
    bqk_e = nc.declare_dram_parameter("bqk", [128, 8], dt.float32, isOutput=False)
    bvr_e = nc.declare_dram_parameter("bvr", [1, CL], dt.bfloat16, isOutput=False)
    wp_e = nc.declare_dram_parameter("wproj", [C, CL], dt.bfloat16, isOutput=False)
    bp_e = nc.declare_dram_parameter("bproj", [128, 4], dt.float32, isOutput=False)
    out_e = nc.declare_dram_parameter("out", [CL, T], dt.float32, isOutput=True)

    RG = [[0, 1], [2, 3], [4, 5], [6, 7]]
    LAST = 3            # last-processed chunk gets the low-latency tail path

    with TileContext(nc) as tc, nc.allow_low_precision("bf16 matmul inputs by design"):
        with ExitStack() as top:
            p_cst = top.enter_context(tc.tile_pool(name="cst", bufs=1))
            p_kt = top.enter_context(tc.tile_pool(name="kt", bufs=4))
            p_v = top.enter_context(tc.tile_pool(name="v", bufs=16))
            p_wq = top.enter_context(tc.tile_pool(name="wq", bufs=8))
            p_wp = top.enter_context(tc.tile_pool(name="wp", bufs=8))
            p_xt = top.enter_context(tc.tile_pool(name="xt", bufs=9))
            pp_wk = top.enter_context(tc.tile_pool(name="ppwk", bufs=2, space="PSUM"))
            pp_q = top.enter_context(tc.tile_pool(name="ppq", bufs=1, space="PSUM"))

            kt_sb = [p_kt.tile([128, T], dt.bfloat16, tag="kt", name=f"ktt{i}")
                     for i in range(4)]
            v_sb = [p_v.tile([128, 8 * 65], dt.bfloat16, tag="v", name=f"vt{i}")
                    for i in range(NT)]

            # ---------------- prologue: k^T and v for all chunks ----------------
            with ExitStack() as pctx:
                p_wkv = pctx.enter_context(tc.tile_pool(name="wkv", bufs=8))
                pp_v = pctx.enter_context(tc.tile_pool(name="ppv", bufs=2, space="PSUM"))
                # first matmul chain needs x(chunk0, all c) + the mk=0 column
                # slice of every k-weight tile: emit those DMAs first so the
                # PE starts as early as possible.
                wkv_sb = []
                xts0 = []
                for c in range(8):
                    wt = p_wkv.tile([128, 1024], dt.bfloat16, tag="wkv", name=f"wkvt{c}")
                    nc.sync.dma_start(wt[:, 0:128], wqk_e[c * 128:(c + 1) * 128, CL:CL + 128])
                    wkv_sb.append(wt)
                    xt = p_xt.tile([128, 512], dt.bfloat16, tag="xt", name=f"x0_{c}")
                    nc.sync.dma_start(xt[:], xT_e[c * 128:(c + 1) * 128, 0:512])
                    xts0.append(xt)
                ones_f = p_cst.tile([128, 128], dt.bfloat16)
                nc.gpsimd.memset(ones_f[:], 1.0)
                ones_row = p_cst.tile([1, 128], dt.bfloat16)
                nc.vector.tensor_copy(ones_row[:], ones_f[0:1, :])
                bqk_sb = p_cst.tile([128, 8], dt.float32)
                nc.sync.dma_start(bqk_sb[:], bqk_e[:])
                bp_sb = p_cst.tile([128, 4], dt.float32)
                nc.sync.dma_start(bp_sb[:], bp_e[:])
                bvr_sb = p_cst.tile([1, CL], dt.bfloat16)
                nc.sync.dma_start(bvr_sb[:], bvr_e[:])
                for c in range(8):
                    nc.sync.dma_start(wkv_sb[c][:, 128:512],
                                      wqk_e[c * 128:(c + 1) * 128, CL + 128:2 * CL])
                for c in range(8):
                    nc.sync.dma_start(wkv_sb[c][:, 512:1024], wv_e[c * 128:(c + 1) * 128, :])

                for n in range(NQ):
                    if n == 0:
                        xts = xts0
                    else:
                        xts = []
                        for c in range(8):
                            xt = p_xt.tile([128, 512], dt.bfloat16, tag="xt", name=f"x{n}_{c}")
                            nc.sync.dma_start(xt[:], xT_e[c * 128:(c + 1) * 128, n * 512:(n + 1) * 512])
                            xts.append(xt)
                    for mk in range(4):
                        ps_t = pp_wk.tile([128, 1024], dt.float32, tag="wk")
                        for c in range(8):
                            nc.tensor.matmul(ps_t[:, 0:512], wkv_sb[c][:, mk * 128:(mk + 1) * 128],
                                             xts[c][:], start=(c == 0), stop=(c == 7))
                        nc.scalar.activation(kt_sb[mk][:, n * 512:(n + 1) * 512], ps_t[:, 0:512],
                                             AF.Identity, bias=bqk_sb[:, 4 + mk:5 + mk])
                    for tv in range(4):
                        ps_v = pp_v.tile([128, 512], dt.float32, tag="pv")
                        for c in range(8):
                            nc.tensor.matmul(ps_v[:], xts[c][:, tv * 128:(tv + 1) * 128],
                                             wkv_sb[c][:, 512:1024], start=(c == 0), stop=False)
                        nc.tensor.matmul(ps_v[:], ones_row[:], bvr_sb[:], start=False, stop=True)
                        vt = v_sb[n * 4 + tv]
                        nc.scalar.activation(
                            vt[:].rearrange("p (h s) -> p h s", s=65)[:, :, 0:64],
                            ps_v[:].rearrange("p (h s) -> p h s", s=64),
                            AF.Copy)
                        nc.vector.tensor_copy(vt[:, 64:520:65], ones_f[:, 0:8])

            # ---------------- main loop ----------------
            wq_sb = []
            for c in range(8):
                wqt = p_wq.tile([128, CL], dt.bfloat16, tag="wq", name=f"wqt{c}")
                nc.sync.dma_start(wqt[:], wqk_e[c * 128:(c + 1) * 128, 0:CL])
                wq_sb.append(wqt)
            wp_sb = []
            for ci in range(8):
                wpt = p_wp.tile([128, CL], dt.bfloat16, tag="wp", name=f"wpt{ci}")
                nc.sync.dma_start(wpt[:], wp_e[ci * 128:(ci + 1) * 128, :])
                wp_sb.append(wpt)

            with ExitStack() as bctx:
                p_q = bctx.enter_context(tc.tile_pool(name="q", bufs=8))
                p_att = bctx.enter_context(tc.tile_pool(name="att", bufs=8))
                p_y = bctx.enter_context(tc.tile_pool(name="yt", bufs=6))
                p_ya = bctx.enter_context(tc.tile_pool(name="ya", bufs=16))
                p_rec = bctx.enter_context(tc.tile_pool(name="rec", bufs=2))
                p_bc = bctx.enter_context(tc.tile_pool(name="bc", bufs=2))
                p_out = bctx.enter_context(tc.tile_pool(name="osb", bufs=2))
                pp_y = bctx.enter_context(tc.tile_pool(name="ppy", bufs=2, space="PSUM"))
                pp_bc = bctx.enter_context(tc.tile_pool(name="ppbc", bufs=1, space="PSUM"))
                p_dram = bctx.enter_context(tc.tile_pool(name="ccd", bufs=4, space="DRAM"))

                q_tiles = {}      # n -> [4 tiles of [128, 512]]; ("x", n) -> x tiles
                yt_tiles = {}     # n -> [4 tiles]
                ya_tiles = {}     # n -> [8 tiles] gathered y^T in global row order
                pair_store = {}   # (n, hp, j) -> (m0, m1, {h: (a_t, q0, q1)})
                ypss_store = {}   # (n, hp) -> {h: y_ps}
                cc_tiles = {}

                def emit_q_slice(n, mq):
                    if mq == 0:
                        xts = []
                        for c in range(8):
                            xt = p_xt.tile([128, 512], dt.bfloat16, tag="xt", name=f"xq{n}_{c}")
                            nc.sync.dma_start(xt[:], xT_e[c * 128:(c + 1) * 128,
                                                          n * 512:(n + 1) * 512])
                            xts.append(xt)
                        q_tiles[("x", n)] = xts
                        q_tiles[n] = []
                    xts = q_tiles[("x", n)]
                    ps_t = pp_q.tile([128, 512], dt.float32, tag="qv")
                    for c in range(8):
                        nc.tensor.matmul(ps_t[:], wq_sb[c][:, mq * 128:(mq + 1) * 128],
                                         xts[c][:], start=(c == 0), stop=(c == 7))
                    qt = p_q.tile([128, 512], dt.bfloat16, tag="q", name=f"q{n}_{mq}")
                    nc.scalar.activation(qt[:], ps_t[:], AF.Identity, bias=bqk_sb[:, mq:mq + 1])
                    q_tiles[n].append(qt)

                def emit_scores_pair(n, hp, j):
                    h0, h1 = 2 * hp, 2 * hp + 1
                    if j == 0:
                        ypss_store[(n, hp)] = {
                            h: pp_y.tile([128, 512], dt.float32, tag="ypsum",
                                         name=f"yps{n}_{h}")
                            for h in (h0, h1)}
                    m0, m1 = 2 * j, 2 * j + 1
                    r0, r1 = m0 - 4 * n, m1 - 4 * n
                    q0 = 128 * r0 if r0 >= 0 else 0
                    q1 = 128 * r1 if r1 >= 0 else 0
                    entry = {}
                    for h in (h0, h1):
                        base = (h % 2) * 64
                        qt = q_tiles[n][h // 2]
                        kt = kt_sb[h // 2]
                        s_ps = pp_wk.tile([128, 1024], dt.float32, tag="wk")
                        nc.tensor.matmul(
                            s_ps[:, q0:512],
                            kt[base:base + 64, m0 * 128:(m0 + 1) * 128],
                            qt[base:base + 64, q0:512],
                            start=True, stop=True)
                        nc.tensor.matmul(
                            s_ps[:, 512 + q1:1024],
                            kt[base:base + 64, m1 * 128:(m1 + 1) * 128],
                            qt[base:base + 64, q1:512],
                            start=True, stop=True)
                        a_t = p_att.tile([128, 1024], dt.bfloat16, tag="att",
                                         name=f"a{n}_{hp}_{j}_{h}")
                        nc.scalar.activation(a_t[:, q0:1024], s_ps[:, q0:1024],
                                             AF.Exp, scale=float(SCALE))
                        if r0 >= 0:
                            nc.gpsimd.affine_select(
                                out=a_t[:, q0:q0 + 128], in_=a_t[:, q0:q0 + 128],
                                compare_op=mybir.AluOpType.is_ge, fill=0.0, base=0,
                                pattern=[[1, 128]], channel_multiplier=-1)
                        if r1 >= 0:
                            nc.gpsimd.affine_select(
                                out=a_t[:, 512 + q1:512 + q1 + 128],
                                in_=a_t[:, 512 + q1:512 + q1 + 128],
                                compare_op=mybir.AluOpType.is_ge, fill=0.0, base=0,
                                pattern=[[1, 128]], channel_multiplier=-1)
                        entry[h] = (a_t, q0, q1)
                    pair_store[(n, hp, j)] = (m0, m1, entry)

                def emit_avs_pair(n, hp, j):
                    m_max = 4 * n + 4
                    h0, h1 = 2 * hp, 2 * hp + 1
                    y_pss = ypss_store[(n, hp)]
                    m0, m1, entry = pair_store.pop((n, hp, j))
                    for h in (h0, h1):
                        a_t, q0, q1 = entry[h]
                        nc.tensor.matmul(
                            y_pss[h][0:65, q0:512],
                            v_sb[m0][:, h * 65:h * 65 + 65],
                            a_t[:, q0:512],
                            start=(m0 == 0), stop=False)
                        nc.tensor.matmul(
                            y_pss[h][0:65, q1:512],
                            v_sb[m1][:, h * 65:h * 65 + 65],
                            a_t[:, 512 + q1:1024],
                            start=False, stop=(m1 == m_max - 1))

                def emit_norm(n, hp):
                    h0, h1 = 2 * hp, 2 * hp + 1
                    y_pss = ypss_store.pop((n, hp))
                    yt = p_y.tile([128, 512], dt.bfloat16, tag="yt", name=f"yt{n}_{hp}")
                    yt_tiles.setdefault(n, []).append(yt)
                    if n == LAST:
                        cc_in = p_dram.tile([128, 512], dt.bfloat16, tag="ccinL",
                                            name=f"cil{hp}")
                    else:
                        cc_in = cc_tiles[n][0] if hp < 2 else cc_tiles[n][2]
                    for h in (h0, h1):
                        base = (h % 2) * 64
                        rec_s = p_rec.tile([128, 512], dt.float32, tag="recs")
                        rec = p_rec.tile([128, 512], dt.float32, tag="rec")
                        nc.vector.tensor_copy(rec_s[0:1, :], y_pss[h][64:65, :])
                        nc.vector.reciprocal_approx_fast(out=rec[0:1, :], in_=rec_s[0:1, :])
                        bc_sb = p_bc.tile([128, 512], dt.float32)
                        if n == LAST:
                            # latency-critical tail: proven matmul broadcast
                            rec_r = p_rec.tile([128, 512], dt.bfloat16, tag="recs2")
                            nc.vector.tensor_copy(rec_r[0:1, :], rec[0:1, :])
                            bc_ps = pp_bc.tile([64, 512], dt.float32)
                            nc.tensor.matmul(bc_ps[:], ones_row[0:1, 0:64], rec_r[0:1, :],
                                             start=True, stop=True)
                            nc.vector.tensor_copy(bc_sb[0:64, :], bc_ps[:])
                        else:
                            # off the critical path: broadcast on gpsimd, no
                            # PE matmul and no bf16 staging needed
                            nc.gpsimd.partition_broadcast(bc_sb[0:64, :], rec[0:1, :],
                                                          channels=64)
                        nc.vector.tensor_mul(yt[base:base + 64, :], y_pss[h][0:64, :],
                                             bc_sb[0:64, :])
                        row = (0 if n == LAST else (hp % 2) * 128) + base
                        nc.sync.dma_start(cc_in[row:row + 64, :], yt[base:base + 64, :])
                    if n == LAST:
                        cc_out = p_dram.tile([256, 512], dt.bfloat16, tag="ccoutL",
                                             name=f"col{hp}")
                        nc.gpsimd.collective_compute(
                            "AllGather", mybir.AluOpType.bypass,
                            ins=[cc_in[:]], outs=[cc_out[:]], replica_groups=RG)
                        ya = ya_tiles.setdefault(n, [None] * 8)
                        for half, gi in ((0, hp), (1, 4 + hp)):
                            t = p_ya.tile([128, 512], dt.bfloat16, tag="ya",
                                          name=f"ya{n}_{gi}")
                            nc.sync.dma_start(t[:], cc_out[half * 128:(half + 1) * 128, :])
                            ya[gi] = t

                def emit_ag(n, half):
                    # chunks 0..2: AllGather half the chunk's y^T across the
                    # pair. Replica order [even, odd] -> global rows:
                    #   half 0: [0:256] + [512:768]   -> ya tiles 0,1,4,5
                    #   half 1: [256:512] + [768:1024] -> ya tiles 2,3,6,7
                    cc_inA, cc_outA, cc_inB, cc_outB = cc_tiles[n]
                    cc_in, cc_out = (cc_inA, cc_outA) if half == 0 else (cc_inB, cc_outB)
                    nc.gpsimd.collective_compute(
                        "AllGather", mybir.AluOpType.bypass,
                        ins=[cc_in[:]], outs=[cc_out[:]], replica_groups=RG)
                    ya = ya_tiles.setdefault(n, [None] * 8)
                    for i in range(4):
                        gi = [0, 1, 4, 5][i] if half == 0 else [2, 3, 6, 7][i]
                        t = p_ya.tile([128, 512], dt.bfloat16, tag="ya",
                                      name=f"ya{n}_{gi}")
                        nc.sync.dma_start(t[:], cc_out[i * 128:(i + 1) * 128, :])
                        ya[gi] = t

                def emit_c_chunk(n, co):
                    ya = ya_tiles[n]
                    o_ps = pp_wk.tile([128, 1024], dt.float32, tag="wk")
                    for ci in range(8):
                        nc.tensor.matmul(o_ps[:, 0:512], wp_sb[ci][:, co * 128:(co + 1) * 128],
                                         ya[ci][:], start=(ci == 0), stop=(ci == 7))
                    o_sb = p_out.tile([128, 512], dt.float32)
                    nc.scalar.activation(o_sb[:], o_ps[:, 0:512], AF.Identity,
                                         bias=bp_sb[:, co:co + 1])
                    nc.sync.dma_start(out_e[co * 128:(co + 1) * 128, n * 512:(n + 1) * 512], o_sb[:])

                def emit_c_tail(n):
                    # final chunk: three parallel accumulators (2 wk slots +
                    # the q slot, all free by now) run in AG-arrival order so
                    # only the last head pair's AllGather is tail-exposed;
                    # the fourth column chunk streams as a final sweep.
                    ya = ya_tiles[n]
                    o_pss = [
                        pp_wk.tile([128, 1024], dt.float32, tag="wk", name="otA")[:, 0:512],
                        pp_wk.tile([128, 1024], dt.float32, tag="wk", name="otB")[:, 0:512],
                        pp_q.tile([128, 512], dt.float32, tag="qv", name="otC")[:],
                    ]
                    arrival = [0, 4, 1, 5, 2, 6, 3, 7]
                    for i, ci in enumerate(arrival):
                        for co in range(3):
                            nc.tensor.matmul(o_pss[co],
                                             wp_sb[ci][:, co * 128:(co + 1) * 128],
                                             ya[ci][:], start=(i == 0), stop=(i == 7))
                    o_d = pp_y.tile([128, 512], dt.float32, tag="ypsum", name="otD")[:]
                    for i, ci in enumerate(arrival):
                        nc.tensor.matmul(o_d, wp_sb[ci][:, 384:512],
                                         ya[ci][:], start=(i == 0), stop=(i == 7))
                    for co in range(4):
                        o_ps = o_pss[co] if co < 3 else o_d
                        o_sb = p_out.tile([128, 512], dt.float32)
                        nc.scalar.activation(o_sb[:], o_ps, AF.Identity,
                                             bias=bp_sb[:, co:co + 1])
                        nc.sync.dma_start(
                            out_e[co * 128:(co + 1) * 128, n * 512:(n + 1) * 512], o_sb[:])

                def emit_filler(f):
                    if f[0] == "q":
                        emit_q_slice(f[1], f[2])
                    else:
                        emit_c_chunk(f[1], f[2])

                ORD = [0, 1, 2, 3]  # natural B-chunk order
                for step in range(6):
                    bn = ORD[step - 1] if 1 <= step <= 4 else -1
                    qn = ORD[step] if step < NQ else -1
                    cn = ORD[step - 2] if step >= 2 else -1
                    if bn >= 0 and bn != LAST:
                        cc_tiles[bn] = (
                            p_dram.tile([256, 512], dt.bfloat16, tag="ccinA", name=f"cia{bn}"),
                            p_dram.tile([512, 512], dt.bfloat16, tag="ccoutA", name=f"coa{bn}"),
                            p_dram.tile([256, 512], dt.bfloat16, tag="ccinB", name=f"cib{bn}"),
                            p_dram.tile([512, 512], dt.bfloat16, tag="ccoutB", name=f"cob{bn}"),
                        )
                    fillers = []
                    if qn >= 0:
                        fillers += [("q", qn, mq) for mq in range(4)]
                    if cn >= 0 and cn != LAST:
                        fillers += [("c", cn, co) for co in range(4)]
                    if bn < 0:
                        for f in fillers:
                            emit_filler(f)
                        if cn == LAST:
                            emit_c_tail(cn)
                        continue
                    pairs_total = (2 * bn + 2) * 4
                    k = 0
                    fi = 0
                    for hp in range(4):
                        npair = 2 * bn + 2
                        for j in range(npair):
                            emit_scores_pair(bn, hp, j)
                            while fi < len(fillers) and fi * pairs_total < (k + 1) * len(fillers):
                                emit_filler(fillers[fi])
                                fi += 1
                            if j >= 2:
                                emit_avs_pair(bn, hp, j - 2)
                            k += 1
                        emit_avs_pair(bn, hp, npair - 2)
                        emit_avs_pair(bn, hp, npair - 1)
                        emit_norm(bn, hp)
                        if bn != LAST:
                            if hp == 1:
                                emit_ag(bn, 0)
                            elif hp == 3:
                                emit_ag(bn, 1)
                    while fi < len(fillers):
                        emit_filler(fillers[fi])
                        fi += 1

    nc.finalize()
    return nc


def _get_nc():
    if "nc" not in _CACHE:
        _CACHE["nc"] = _build_nc()
    return _CACHE["nc"]


def _make_in_maps(x, W_attn, b_attn, W_proj, b_proj):
    x = np.asarray(x, dtype=np.float32)
    W_attn = np.asarray(W_attn, dtype=np.float32)
    b_attn = np.asarray(b_attn, dtype=np.float32)
    W_proj = np.asarray(W_proj, dtype=np.float32)
    b_proj = np.asarray(b_proj, dtype=np.float32)

    in_maps = []
    for core in range(8):
        b, hg = core // 2, core % 2
        lo, hi = hg * CL, (hg + 1) * CL
        wq = W_attn[:, lo:hi]
        wk = W_attn[:, C + lo:C + hi]
        wv = W_attn[:, 2 * C + lo:2 * C + hi]
        bq = b_attn[lo:hi]
        bk = b_attn[C + lo:C + hi]
        bv = b_attn[2 * C + lo:2 * C + hi]
        bp = b_proj[lo:hi]
        in_maps.append({
            "xT": np.ascontiguousarray(x[b].T).astype(BF16),
            "wqk": np.ascontiguousarray(np.concatenate([wq, wk], axis=1)).astype(BF16),
            "wv": np.ascontiguousarray(wv).astype(BF16),
            "bqk": np.ascontiguousarray(np.concatenate([bq, bk]).reshape(8, 128).T),
            "bvr": np.ascontiguousarray(bv.reshape(1, CL)).astype(BF16),
            "wproj": np.ascontiguousarray(W_proj[:, lo:hi]).astype(BF16),
            "bproj": np.ascontiguousarray(bp.reshape(4, 128).T),
        })
    return in_maps


def _assemble(results):
    out = np.empty((B, T, C), dtype=np.float32)
    outT = np.empty((C, T), dtype=np.float32)
    for b in range(B):
        outT[0:512] = results[2 * b]["out"]
        outT[512:1024] = results[2 * b + 1]["out"]
        out[b] = outT.T
    return out


def run(trace=False, **inputs):
    nc = _get_nc()
    in_maps = _make_in_maps(**inputs)
    kw = {}
    if trace:
        kw = dict(trace=True, trace_cores=[0])
    res = run_bass_kernel_spmd(nc, in_maps, list(range(8)), **kw)
    return _assemble(res.results), res


def kernel(**inputs) -> np.ndarray:
    out, _ = run(trace=False, **inputs)
    return out
